# revision 19
# baseline (speedup 1.0000x reference)
"""KimiSparseMoE Trainium2 kernel (8 NeuronCores, DFF-sharded expert parallel).

Routing structure (provable from the reference algorithm, verified
numerically): the group-limited top-k with the scatter(...,k,1) quirk can
only ever route to experts {0, 1, 2, 8, 16, 24}; experts 0/1 serve every
token, and each token additionally uses exactly 2 of {2, 8, 16, 24}
(chosen by its top-2 groups), with weights = renormalized sigmoid scores.

Parallelization: the 7 dense FFNs (shared + 6 hot experts) are split into
56 chunks of 128 DFF rows. Cores are arranged as TG token-groups x DG
DFF-shards (TG*DG = 8). Core c = DG*g + r processes token half g
(T/TG tokens) through its NCH = 56/DG chunks, so each core holds only
~1/DG of the expert weights (the baseline replicated all of them and was
HBM-bound). Per-chunk partial outputs accumulate in PSUM; a bf16
ReduceScatter over each DFF-group combines partials so core c lands
exactly tokens [128c, 128c+128). The router (f32) is computed on every
core; per-chunk combine coefficients are selected with a per-core
selector matmul so the SPMD program is identical on all cores.
"""

import numpy as np

import concourse.bass as bass
import concourse.mybir as mybir
from concourse.tile import TileContext
from concourse.masks import make_identity
from concourse.bass_utils import run_bass_kernel_spmd

F32 = mybir.dt.float32
BF16 = mybir.dt.bfloat16
AX = mybir.AxisListType.X
ALU = mybir.AluOpType
ACT = mybir.ActivationFunctionType

N_CORES = 8
TG = 2                     # token groups
DG = N_CORES // TG         # DFF-shard groups per token group
T, D, E, DFF = 1024, 2048, 32, 1024
TT = T // TG               # tokens per core
NT = TT // 128             # token tiles per core
KD = D // 128              # contraction tiles over D
NFFN = 7                   # shared + 6 hot experts
NCHG = NFFN * DFF // 128   # 56 global chunks of 128 DFF rows
NCH = NCHG // DG           # chunks per core
HOT = [0, 1, 2, 8, 16, 24]
SCALING = 2.5

MODE = "bf16"              # kept for test.py compat

_MAX_WAITS = 1  # this container's walrus accepts one sem-wait per instruction


def _split_sync_waits(nc):
    for fn in nc.m.functions:
        for blk in fn.blocks:
            old = list(blk.instructions)
            new = []
            changed = False
            for ins in old:
                si = ins.sync_info
                if si is not None and len(si.on_wait) > _MAX_WAITS:
                    waits = list(si.on_wait)
                    keep, rest = waits[:_MAX_WAITS], waits[_MAX_WAITS:]
                    for i in range(0, len(rest), _MAX_WAITS):
                        nop = mybir.InstNoOp(
                            name=nc.get_next_instruction_name(),
                            engine=ins.engine,
                            sync_info=mybir.SyncInfo(
                                on_wait=rest[i : i + _MAX_WAITS], on_update=[]
                            ),
                            bass_nofuse=True,
                        )
                        new.append(nop)
                        changed = True
                    si.on_wait = keep
                new.append(ins)
            if changed:
                blk.instructions = new


def build():
    nc = bass.Bass("TRN2", target_bir_lowering=False, debug=False, num_devices=N_CORES)

    xtf_d = nc.dram_tensor("xtf", [128, KD * TT], F32, kind="ExternalInput")
    xtb_d = nc.dram_tensor("xtb", [128, KD * TT], BF16, kind="ExternalInput")
    gwt_d = nc.dram_tensor("gwt", [128, KD * E], F32, kind="ExternalInput")
    biasr_d = nc.dram_tensor("biasr", [128, NT * E], F32, kind="ExternalInput")
    selmat_d = nc.dram_tensor("selmat", [8, NCH * 128], F32, kind="ExternalInput")
    wg_d = nc.dram_tensor("wg", [NCH, 128, KD * 128], BF16, kind="ExternalInput")
    wu_d = nc.dram_tensor("wu", [NCH, 128, KD * 128], BF16, kind="ExternalInput")
    wd_d = nc.dram_tensor("wd", [NCH, 128, D], BF16, kind="ExternalInput")
    out_d = nc.dram_tensor("out", [128, D], F32, kind="ExternalOutput")

    groups = [
        [TG_g * DG + r for r in range(DG)] for TG_g in range(TG)
    ]

    with TileContext(nc) as tc:
        with (
            tc.sbuf_pool(name="const", bufs=1) as cpool,
            tc.sbuf_pool(name="rt", bufs=1) as rt,
            tc.sbuf_pool(name="wgp", bufs=2) as wgp,
            tc.sbuf_pool(name="wup", bufs=2) as wup,
            tc.sbuf_pool(name="silup", bufs=2) as silup,
            tc.sbuf_pool(name="stg", bufs=3) as stg,
            tc.sbuf_pool(name="fin", bufs=1) as fin,
            tc.psum_pool(name="gup", bufs=2) as gup,
            tc.psum_pool(name="misc", bufs=1) as miscp,
            tc.psum_pool(name="outp", bufs=2) as outp,
            tc.tile_pool(name="dram", bufs=1, space="DRAM") as dram,
        ):
            # ---- persistent tiles ----
            xtb_sb = cpool.tile([128, KD * TT], BF16)
            nc.sync.dma_start(xtb_sb, xtb_d[:, :])
            xtf_sb = cpool.tile([128, KD * TT], F32)
            gwt_sb = cpool.tile([128, KD * E], F32)
            biasr_sb = cpool.tile([128, NT * E], F32)
            selmat_sb = cpool.tile([8, NCH * 128], F32)
            identity = cpool.tile([128, 128], F32)
            hraw = cpool.tile([128, NCH * TT], BF16)
            hsc = cpool.tile([128, NCH * TT], BF16)
            wd_sb = cpool.tile([128, NCH * D], BF16)
            coeff_pack = cpool.tile([128, 128], F32)
            ct_stage = cpool.tile([8, NT * 128], F32)

            rs_in = [
                dram.tile([TT, D // 2], BF16, tag=f"i{h}", name=f"rs_in{h}")
                for h in range(2)
            ]
            rs_out = [
                dram.tile([128, D // 2], BF16, tag=f"o{h}", name=f"rs_out{h}")
                for h in range(2)
            ]

            def emit_router():
                gates_ps = miscp.tile([128, 512], F32, tag="gates")
                for tt in range(NT):
                    for k in range(KD):
                        nc.tensor.matmul(
                            gates_ps[:, 32 * tt : 32 * tt + 32],
                            lhsT=xtf_sb[:, k * TT + 128 * tt : k * TT + 128 * tt + 128],
                            rhs=gwt_sb[:, 32 * k : 32 * k + 32],
                            start=(k == 0),
                            stop=(k == KD - 1),
                        )
                s_all = rt.tile([128, NT * E], F32, tag="s_all")
                nc.scalar.activation(s_all, gates_ps[:, : NT * E], ACT.Sigmoid)
                sb_all = rt.tile([128, NT * E], F32, tag="sb_all")
                nc.vector.tensor_add(sb_all, s_all, biasr_sb)
                nc.gpsimd.memset(coeff_pack, 0.0)
                for tt in range(NT):
                    nc.gpsimd.memset(coeff_pack[:, 8 * tt + 6 : 8 * tt + 7], 1.0)
                for tt in range(NT):
                    s = s_all[:, E * tt : E * tt + E]
                    sb = sb_all[:, E * tt : E * tt + E]
                    gs = rt.tile([128, 4], F32, tag="gs")
                    for g in range(4):
                        grp = sb[:, 8 * g : 8 * g + 8]
                        m1 = rt.tile([128, 1], F32, tag="m1")
                        nc.vector.reduce_max(m1, grp, AX)
                        eq = rt.tile([128, 8], F32, tag="eq")
                        nc.vector.tensor_scalar(eq, grp, m1, None, ALU.is_equal)
                        t2 = rt.tile([128, 8], F32, tag="t2")
                        nc.vector.scalar_tensor_tensor(
                            t2, eq, -1e30, grp, ALU.mult, ALU.add
                        )
                        m2 = rt.tile([128, 1], F32, tag="m2")
                        nc.vector.reduce_max(m2, t2, AX)
                        nc.vector.tensor_tensor(gs[:, g : g + 1], m1, m2, ALU.add)
                    g1 = rt.tile([128, 1], F32, tag="g1")
                    eq1 = rt.tile([128, 4], F32, tag="eq1")
                    gsm = rt.tile([128, 4], F32, tag="gsm")
                    g2 = rt.tile([128, 1], F32, tag="g2")
                    eq2 = rt.tile([128, 4], F32, tag="eq2")
                    gmask = rt.tile([128, 4], F32, tag="gmask")
                    nc.vector.reduce_max(g1, gs, AX)
                    nc.vector.tensor_scalar(eq1, gs, g1, None, ALU.is_equal)
                    nc.vector.scalar_tensor_tensor(
                        gsm, eq1, -1e30, gs, ALU.mult, ALU.add
                    )
                    nc.vector.reduce_max(g2, gsm, AX)
                    nc.vector.tensor_scalar(eq2, gsm, g2, None, ALU.is_equal)
                    nc.vector.tensor_add(gmask, eq1, eq2)

                    hs = rt.tile([128, 6], F32, tag="hs")
                    nc.vector.tensor_copy(hs[:, 0:3], s[:, 0:3])
                    nc.vector.tensor_copy(hs[:, 3:4], s[:, 8:9])
                    nc.vector.tensor_copy(hs[:, 4:5], s[:, 16:17])
                    nc.vector.tensor_copy(hs[:, 5:6], s[:, 24:25])
                    nc.vector.tensor_tensor(hs[:, 2:6], hs[:, 2:6], gmask, ALU.mult)
                    denom = rt.tile([128, 1], F32, tag="denom")
                    nc.vector.reduce_sum(denom, hs, AX)
                    rec = rt.tile([128, 1], F32, tag="rec")
                    nc.vector.reciprocal(rec, denom)
                    nc.vector.tensor_scalar(
                        coeff_pack[:, 8 * tt : 8 * tt + 6],
                        hs,
                        rec,
                        SCALING,
                        ALU.mult,
                        ALU.mult,
                    )
                # transpose coeff_pack -> rows (8*tt + e), then select per-chunk
                # coefficient rows with the per-core selector matmul.
                # transpose each tile's [128 tok, 8] coeff block to [8, 128 tok]
                # so expert-row e lands at partition e for every tile.
                make_identity(nc, identity)
                tp = miscp.tile([128, 512], F32, tag="gates")
                for tt in range(NT):
                    nc.tensor.transpose(
                        tp[:8, 128 * tt : 128 * tt + 128],
                        coeff_pack[:, 8 * tt : 8 * tt + 8],
                        identity,
                    )
                nc.vector.tensor_copy(ct_stage, tp[:8, : NT * 128])

            # ---- phase 1: H = silu(x Wg^T) * (x Wu^T) per chunk ----
            for j in range(NCH):
                if j == 3:
                    nc.sync.dma_start(xtf_sb, xtf_d[:, :])
                    nc.sync.dma_start(gwt_sb, gwt_d[:, :])
                    nc.sync.dma_start(biasr_sb, biasr_d[:, :])
                    nc.sync.dma_start(selmat_sb, selmat_d[:, :])
                if j == 4:
                    emit_router()
                wgc = wgp.tile([128, KD * 128], BF16, tag="wg")
                nc.sync.dma_start(wgc, wg_d[j])
                wuc = wup.tile([128, KD * 128], BF16, tag="wu")
                nc.sync.dma_start(wuc, wu_d[j])
                nc.sync.dma_start(wd_sb[:, j * D : (j + 1) * D], wd_d[j])

                G = gup.tile([128, TT], F32, tag="g")
                for k in range(KD):
                    nc.tensor.matmul(
                        G,
                        lhsT=wgc[:, 128 * k : 128 * (k + 1)],
                        rhs=xtb_sb[:, TT * k : TT * (k + 1)],
                        start=(k == 0),
                        stop=(k == KD - 1),
                    )
                silu_t = silup.tile([128, TT], F32, tag="s")
                nc.scalar.activation(silu_t, G, ACT.Silu)
                U = gup.tile([128, TT], F32, tag="u")
                for k in range(KD):
                    nc.tensor.matmul(
                        U,
                        lhsT=wuc[:, 128 * k : 128 * (k + 1)],
                        rhs=xtb_sb[:, TT * k : TT * (k + 1)],
                        start=(k == 0),
                        stop=(k == KD - 1),
                    )
                nc.vector.tensor_tensor(
                    hraw[:, j * TT : (j + 1) * TT], silu_t, U, ALU.mult
                )

            # ---- phase 1.5: scale H rows by per-chunk combine coefficients.
            # cb[p, t] = coeff[t, ffn(chunk j)] for every partition p, via a
            # matmul with the selector column replicated across partitions.
            for j in range(NCH):
                cb_ps = miscp.tile([128, 512], F32, tag="cb")
                for tt in range(NT):
                    nc.tensor.matmul(
                        cb_ps[:, 128 * tt : 128 * tt + 128],
                        lhsT=selmat_sb[:8, 128 * j : 128 * j + 128],
                        rhs=ct_stage[:8, 128 * tt : 128 * tt + 128],
                        start=True,
                        stop=True,
                    )
                nc.vector.tensor_tensor(
                    hsc[:, j * TT : (j + 1) * TT],
                    hraw[:, j * TT : (j + 1) * TT],
                    cb_ps[:, :TT],
                    ALU.mult,
                )

            # ---- phase 2: down-proj partials + ReduceScatter per D-half ----
            for hb in range(2):
                for tt in range(NT):
                    for q2 in range(2):
                        O = outp.tile([128, 512], F32, tag="o")
                        col0 = 1024 * hb + 512 * q2
                        for j in range(NCH):
                            nc.tensor.matmul(
                                O,
                                lhsT=hsc[:, j * TT + 128 * tt : j * TT + 128 * tt + 128],
                                rhs=wd_sb[:, j * D + col0 : j * D + col0 + 512],
                                start=(j == 0),
                                stop=(j == NCH - 1),
                            )
                        ob = stg.tile([128, 512], BF16, tag="ob")
                        nc.vector.tensor_copy(ob, O)
                        nc.sync.dma_start(
                            rs_in[hb][
                                128 * tt : 128 * tt + 128, 512 * q2 : 512 * q2 + 512
                            ],
                            ob,
                        )
                nc.gpsimd.collective_compute(
                    "ReduceScatter",
                    ALU.add,
                    replica_groups=groups,
                    ins=[rs_in[hb].opt()],
                    outs=[rs_out[hb].opt()],
                )
                fo_b = fin.tile([128, D // 2], BF16, tag=f"fb{hb}")
                nc.sync.dma_start(fo_b, rs_out[hb][:, :])
                fo_f = fin.tile([128, D // 2], F32, tag=f"ff{hb}")
                nc.vector.tensor_copy(fo_f, fo_b)
                nc.sync.dma_start(out_d[:, 1024 * hb : 1024 * hb + 1024], fo_f)

    _split_sync_waits(nc)
    return nc


def _pack_sbuf16(mat_t, cols, np_dt):
    """[D_rows, cols] (row-major, D_rows = 128*K) -> SBUF image [128, K*cols]."""
    rows = mat_t.shape[0]
    k = rows // 128
    return (
        np.ascontiguousarray(mat_t)
        .reshape(k, 128, cols)
        .transpose(1, 0, 2)
        .reshape(128, k * cols)
        .astype(np_dt, copy=False)
    )


def _pack_inputs(x, gate_w, bias, Wg, Wu, Wd, sWg, sWu, sWd):
    import ml_dtypes

    bf16 = ml_dtypes.bfloat16
    x = np.asarray(x, np.float32)
    gate_w = np.asarray(gate_w, np.float32)
    bias = np.asarray(bias, np.float32)
    Wg, Wu, Wd = (np.asarray(a, np.float32) for a in (Wg, Wu, Wd))
    sWg, sWu, sWd = (np.asarray(a, np.float32) for a in (sWg, sWu, sWd))

    ffn = [(sWg, sWu, sWd)] + [(Wg[e], Wu[e], Wd[e]) for e in HOT]
    wg_pack = np.empty((NCHG, 128, KD * 128), bf16)
    wu_pack = np.empty((NCHG, 128, KD * 128), bf16)
    wd_pack = np.empty((NCHG, 128, D), bf16)
    for f, (wgf, wuf, wdf) in enumerate(ffn):
        wgT = np.ascontiguousarray(wgf.T)  # [D, DFF]
        wuT = np.ascontiguousarray(wuf.T)
        wdT = np.ascontiguousarray(wdf.T)  # [DFF, D]
        for p in range(DFF // 128):
            ch = f * (DFF // 128) + p
            wg_pack[ch] = _pack_sbuf16(wgT[:, 128 * p : 128 * (p + 1)], 128, bf16)
            wu_pack[ch] = _pack_sbuf16(wuT[:, 128 * p : 128 * (p + 1)], 128, bf16)
            wd_pack[ch] = wdT[128 * p : 128 * (p + 1), :].astype(bf16)

    gwt = _pack_sbuf16(np.ascontiguousarray(gate_w.T), E, np.float32)
    biasr = np.broadcast_to(np.tile(bias, NT), (128, NT * E)).astype(np.float32)

    in_maps = []
    for c in range(N_CORES):
        g, r = c // DG, c % DG
        xh = np.ascontiguousarray(x[TT * g : TT * (g + 1)].T)  # [D, TT]
        sel = np.zeros((8, NCH * 128), np.float32)
        for j in range(NCH):
            f = (NCH * r + j) // (DFF // 128)
            sel[6 if f == 0 else f - 1, 128 * j : 128 * (j + 1)] = 1.0
        in_maps.append(
            {
                "xtf": _pack_sbuf16(xh, TT, np.float32),
                "xtb": _pack_sbuf16(xh, TT, bf16),
                "gwt": gwt,
                "biasr": biasr,
                "selmat": sel,
                "wg": wg_pack[NCH * r : NCH * (r + 1)],
                "wu": wu_pack[NCH * r : NCH * (r + 1)],
                "wd": wd_pack[NCH * r : NCH * (r + 1)],
            }
        )
    return in_maps


def run(inputs, mode=MODE, trace=False):
    nc = build()
    in_maps = _pack_inputs(**inputs)
    res = run_bass_kernel_spmd(
        nc, in_maps, core_ids=list(range(N_CORES)), trace=trace
    )
    out = np.concatenate(
        [res.results[c]["out"].astype(np.float32) for c in range(N_CORES)], axis=0
    )
    return out, res


def kernel(**inputs):
    out, _ = run(inputs, trace=False)
    return out


# revision 24
# speedup vs baseline: 1.7246x; 1.7246x over previous
"""KimiSparseMoE Trainium2 kernel (8 NeuronCores, DFF-sharded expert parallel).

Routing structure (provable from the reference algorithm, verified
numerically): the group-limited top-k with the scatter(...,k,1) quirk can
only ever route to experts {0, 1, 2, 8, 16, 24}; experts 0/1 serve every
token, and each token additionally uses exactly 2 of {2, 8, 16, 24}
(chosen by its top-2 groups), with weights = renormalized sigmoid scores.

Parallelization: the 7 dense FFNs (shared + 6 hot experts) are split into
56 chunks of 128 DFF rows. Cores are arranged as TG token-groups x DG
DFF-shards (TG*DG = 8). Core c = DG*g + r processes token half g
(T/TG tokens) through its NCH = 56/DG chunks, so each core holds only
~1/DG of the expert weights (the baseline replicated all of them and was
HBM-bound). Per-chunk partial outputs accumulate in PSUM; a bf16
ReduceScatter over each DFF-group combines partials so core c lands
exactly tokens [128c, 128c+128). The router (f32) is computed on every
core; per-chunk combine coefficients are selected with a per-core
selector matmul so the SPMD program is identical on all cores.
"""

import numpy as np

import concourse.bass as bass
import concourse.mybir as mybir
from concourse.tile import TileContext
from concourse.masks import make_identity
from concourse.bass_utils import run_bass_kernel_spmd

F32 = mybir.dt.float32
BF16 = mybir.dt.bfloat16
AX = mybir.AxisListType.X
ALU = mybir.AluOpType
ACT = mybir.ActivationFunctionType

N_CORES = 8
TG = 2                     # token groups
DG = N_CORES // TG         # DFF-shard groups per token group
T, D, E, DFF = 1024, 2048, 32, 1024
TT = T // TG               # tokens per core
NT = TT // 128             # token tiles per core
KD = D // 128              # contraction tiles over D
NFFN = 7                   # shared + 6 hot experts
NCHG = NFFN * DFF // 128   # 56 global chunks of 128 DFF rows
NCH = NCHG // DG           # chunks per core
HOT = [0, 1, 2, 8, 16, 24]
SCALING = 2.5

MODE = "bf16"              # kept for test.py compat

_MAX_WAITS = 1  # this container's walrus accepts one sem-wait per instruction


def _split_sync_waits(nc):
    for fn in nc.m.functions:
        for blk in fn.blocks:
            old = list(blk.instructions)
            new = []
            changed = False
            for ins in old:
                si = ins.sync_info
                if si is not None and len(si.on_wait) > _MAX_WAITS:
                    waits = list(si.on_wait)
                    keep, rest = waits[:_MAX_WAITS], waits[_MAX_WAITS:]
                    for i in range(0, len(rest), _MAX_WAITS):
                        nop = mybir.InstNoOp(
                            name=nc.get_next_instruction_name(),
                            engine=ins.engine,
                            sync_info=mybir.SyncInfo(
                                on_wait=rest[i : i + _MAX_WAITS], on_update=[]
                            ),
                            bass_nofuse=True,
                        )
                        new.append(nop)
                        changed = True
                    si.on_wait = keep
                new.append(ins)
            if changed:
                blk.instructions = new


def build():
    nc = bass.Bass("TRN2", target_bir_lowering=False, debug=False, num_devices=N_CORES)

    xtf_d = nc.dram_tensor("xtf", [128, KD * TT], F32, kind="ExternalInput")
    xtb_d = nc.dram_tensor("xtb", [128, KD * TT], BF16, kind="ExternalInput")
    gwt_d = nc.dram_tensor("gwt", [128, KD * E], F32, kind="ExternalInput")
    biasr_d = nc.dram_tensor("biasr", [128, NT * E], F32, kind="ExternalInput")
    selmat_d = nc.dram_tensor("selmat", [8, NCH * 128], BF16, kind="ExternalInput")
    wg_d = nc.dram_tensor("wg", [NCH, 128, KD * 128], BF16, kind="ExternalInput")
    wu_d = nc.dram_tensor("wu", [NCH, 128, KD * 128], BF16, kind="ExternalInput")
    wd_d = nc.dram_tensor("wd", [NCH, 128, D], BF16, kind="ExternalInput")
    out_d = nc.dram_tensor("out", [128, D], F32, kind="ExternalOutput")

    groups = [
        [TG_g * DG + r for r in range(DG)] for TG_g in range(TG)
    ]

    with TileContext(nc) as tc:
        with (
            tc.sbuf_pool(name="const", bufs=1) as cpool,
            tc.sbuf_pool(name="rt", bufs=1) as rt,
            tc.sbuf_pool(name="wgp", bufs=2) as wgp,
            tc.sbuf_pool(name="wup", bufs=2) as wup,
            tc.sbuf_pool(name="silup", bufs=2) as silup,
            tc.sbuf_pool(name="stg", bufs=3) as stg,
            tc.sbuf_pool(name="fin", bufs=1) as fin,
            tc.psum_pool(name="gup", bufs=2) as gup,
            tc.psum_pool(name="misc", bufs=1) as miscp,
            tc.psum_pool(name="outp", bufs=2) as outp,
            tc.tile_pool(name="dram", bufs=1, space="DRAM") as dram,
        ):
            # ---- persistent tiles ----
            xtb_sb = cpool.tile([128, KD * TT], BF16)
            nc.sync.dma_start(xtb_sb, xtb_d[:, :])
            xtf_sb = cpool.tile([128, KD * TT], F32)
            gwt_sb = cpool.tile([128, KD * E], F32)
            biasr_sb = cpool.tile([128, NT * E], F32)
            selmat_sb = cpool.tile([8, NCH * 128], BF16)
            identity = cpool.tile([128, 128], F32)
            hraw = cpool.tile([128, NCH * TT], BF16)
            hsc = cpool.tile([128, NCH * TT], BF16)
            wd_sb = cpool.tile([128, NCH * D], BF16)
            coeff_pack = cpool.tile([128, 128], F32)
            ct_stage = cpool.tile([8, NT * 128], BF16)

            rs_in = [
                dram.tile([TT, D // 2], BF16, tag=f"i{h}", name=f"rs_in{h}")
                for h in range(2)
            ]
            rs_out = [
                dram.tile([128, D // 2], BF16, tag=f"o{h}", name=f"rs_out{h}")
                for h in range(2)
            ]

            def emit_router():
                gates_ps = miscp.tile([128, 512], F32, tag="gates")
                for tt in range(NT):
                    for k in range(KD):
                        nc.tensor.matmul(
                            gates_ps[:, 32 * tt : 32 * tt + 32],
                            lhsT=xtf_sb[:, k * TT + 128 * tt : k * TT + 128 * tt + 128],
                            rhs=gwt_sb[:, 32 * k : 32 * k + 32],
                            start=(k == 0),
                            stop=(k == KD - 1),
                        )
                s_all = rt.tile([128, NT * E], F32, tag="s_all")
                nc.scalar.activation(s_all, gates_ps[:, : NT * E], ACT.Sigmoid)
                sb_all = rt.tile([128, NT * E], F32, tag="sb_all")
                nc.vector.tensor_add(sb_all, s_all, biasr_sb)
                nc.gpsimd.memset(coeff_pack, 0.0)
                for tt in range(NT):
                    nc.gpsimd.memset(coeff_pack[:, 8 * tt + 6 : 8 * tt + 7], 1.0)
                for tt in range(NT):
                    s = s_all[:, E * tt : E * tt + E]
                    sb = sb_all[:, E * tt : E * tt + E]
                    gs = rt.tile([128, 4], F32, tag="gs")
                    for g in range(4):
                        grp = sb[:, 8 * g : 8 * g + 8]
                        m1 = rt.tile([128, 1], F32, tag="m1")
                        nc.vector.reduce_max(m1, grp, AX)
                        eq = rt.tile([128, 8], F32, tag="eq")
                        nc.vector.tensor_scalar(eq, grp, m1, None, ALU.is_equal)
                        t2 = rt.tile([128, 8], F32, tag="t2")
                        nc.vector.scalar_tensor_tensor(
                            t2, eq, -1e30, grp, ALU.mult, ALU.add
                        )
                        m2 = rt.tile([128, 1], F32, tag="m2")
                        nc.vector.reduce_max(m2, t2, AX)
                        nc.vector.tensor_tensor(gs[:, g : g + 1], m1, m2, ALU.add)
                    g1 = rt.tile([128, 1], F32, tag="g1")
                    eq1 = rt.tile([128, 4], F32, tag="eq1")
                    gsm = rt.tile([128, 4], F32, tag="gsm")
                    g2 = rt.tile([128, 1], F32, tag="g2")
                    eq2 = rt.tile([128, 4], F32, tag="eq2")
                    gmask = rt.tile([128, 4], F32, tag="gmask")
                    nc.vector.reduce_max(g1, gs, AX)
                    nc.vector.tensor_scalar(eq1, gs, g1, None, ALU.is_equal)
                    nc.vector.scalar_tensor_tensor(
                        gsm, eq1, -1e30, gs, ALU.mult, ALU.add
                    )
                    nc.vector.reduce_max(g2, gsm, AX)
                    nc.vector.tensor_scalar(eq2, gsm, g2, None, ALU.is_equal)
                    nc.vector.tensor_add(gmask, eq1, eq2)

                    hs = rt.tile([128, 6], F32, tag="hs")
                    nc.vector.tensor_copy(hs[:, 0:3], s[:, 0:3])
                    nc.vector.tensor_copy(hs[:, 3:4], s[:, 8:9])
                    nc.vector.tensor_copy(hs[:, 4:5], s[:, 16:17])
                    nc.vector.tensor_copy(hs[:, 5:6], s[:, 24:25])
                    nc.vector.tensor_tensor(hs[:, 2:6], hs[:, 2:6], gmask, ALU.mult)
                    denom = rt.tile([128, 1], F32, tag="denom")
                    nc.vector.reduce_sum(denom, hs, AX)
                    rec = rt.tile([128, 1], F32, tag="rec")
                    nc.vector.reciprocal(rec, denom)
                    nc.vector.tensor_scalar(
                        coeff_pack[:, 8 * tt : 8 * tt + 6],
                        hs,
                        rec,
                        SCALING,
                        ALU.mult,
                        ALU.mult,
                    )
                # transpose coeff_pack -> rows (8*tt + e), then select per-chunk
                # coefficient rows with the per-core selector matmul.
                # transpose each tile's [128 tok, 8] coeff block to [8, 128 tok]
                # so expert-row e lands at partition e for every tile.
                make_identity(nc, identity)
                tp = miscp.tile([128, 512], F32, tag="gates")
                for tt in range(NT):
                    nc.tensor.transpose(
                        tp[:8, 128 * tt : 128 * tt + 128],
                        coeff_pack[:, 8 * tt : 8 * tt + 8],
                        identity,
                    )
                nc.vector.tensor_copy(ct_stage, tp[:8, : NT * 128])

            # ---- phase 1: H = silu(x Wg^T) * (x Wu^T) per chunk ----
            for j in range(NCH):
                if j == 3:
                    nc.sync.dma_start(xtf_sb, xtf_d[:, :])
                    nc.sync.dma_start(gwt_sb, gwt_d[:, :])
                    nc.sync.dma_start(biasr_sb, biasr_d[:, :])
                    nc.sync.dma_start(selmat_sb, selmat_d[:, :])
                if j == 4:
                    emit_router()
                wgc = wgp.tile([128, KD * 128], BF16, tag="wg")
                nc.sync.dma_start(wgc, wg_d[j])
                wuc = wup.tile([128, KD * 128], BF16, tag="wu")
                nc.sync.dma_start(wuc, wu_d[j])
                nc.sync.dma_start(wd_sb[:, j * D : (j + 1) * D], wd_d[j])

                G = gup.tile([128, TT], F32, tag="g")
                for k in range(KD):
                    nc.tensor.matmul(
                        G,
                        lhsT=wgc[:, 128 * k : 128 * (k + 1)],
                        rhs=xtb_sb[:, TT * k : TT * (k + 1)],
                        start=(k == 0),
                        stop=(k == KD - 1),
                    )
                silu_t = silup.tile([128, TT], F32, tag="s")
                nc.scalar.activation(silu_t, G, ACT.Silu)
                U = gup.tile([128, TT], F32, tag="u")
                for k in range(KD):
                    nc.tensor.matmul(
                        U,
                        lhsT=wuc[:, 128 * k : 128 * (k + 1)],
                        rhs=xtb_sb[:, TT * k : TT * (k + 1)],
                        start=(k == 0),
                        stop=(k == KD - 1),
                    )
                nc.vector.tensor_tensor(
                    hraw[:, j * TT : (j + 1) * TT], silu_t, U, ALU.mult
                )

            # ---- phase 1.5: scale H rows by per-chunk combine coefficients.
            # cb[p, t] = coeff[t, ffn(chunk j)] for every partition p, via a
            # matmul with the selector column replicated across partitions.
            for j in range(NCH):
                cb_ps = miscp.tile([128, 512], F32, tag="cb")
                nc.tensor.matmul(
                    cb_ps[:, :TT],
                    lhsT=selmat_sb[:8, 128 * j : 128 * j + 128],
                    rhs=ct_stage[:8, :TT],
                    start=True,
                    stop=True,
                )
                nc.vector.tensor_tensor(
                    hsc[:, j * TT : (j + 1) * TT],
                    hraw[:, j * TT : (j + 1) * TT],
                    cb_ps[:, :TT],
                    ALU.mult,
                )

            # ---- phase 2: down-proj partials + ReduceScatter per D-half ----
            for hb in range(2):
                for tt in range(NT):
                    for q2 in range(2):
                        O = outp.tile([128, 512], F32, tag="o")
                        col0 = 1024 * hb + 512 * q2
                        for j in range(NCH):
                            nc.tensor.matmul(
                                O,
                                lhsT=hsc[:, j * TT + 128 * tt : j * TT + 128 * tt + 128],
                                rhs=wd_sb[:, j * D + col0 : j * D + col0 + 512],
                                start=(j == 0),
                                stop=(j == NCH - 1),
                            )
                        ob = stg.tile([128, 512], BF16, tag="ob")
                        nc.vector.tensor_copy(ob, O)
                        nc.sync.dma_start(
                            rs_in[hb][
                                128 * tt : 128 * tt + 128, 512 * q2 : 512 * q2 + 512
                            ],
                            ob,
                        )
                nc.gpsimd.collective_compute(
                    "ReduceScatter",
                    ALU.add,
                    replica_groups=groups,
                    ins=[rs_in[hb].opt()],
                    outs=[rs_out[hb].opt()],
                )
                fo_b = fin.tile([128, D // 2], BF16, tag=f"fb{hb}")
                nc.sync.dma_start(fo_b, rs_out[hb][:, :])
                fo_f = fin.tile([128, D // 2], F32, tag=f"ff{hb}")
                nc.vector.tensor_copy(fo_f, fo_b)
                nc.sync.dma_start(out_d[:, 1024 * hb : 1024 * hb + 1024], fo_f)

    _split_sync_waits(nc)
    return nc


def _pack_sbuf16(mat_t, cols, np_dt):
    """[D_rows, cols] (row-major, D_rows = 128*K) -> SBUF image [128, K*cols]."""
    rows = mat_t.shape[0]
    k = rows // 128
    return (
        np.ascontiguousarray(mat_t)
        .reshape(k, 128, cols)
        .transpose(1, 0, 2)
        .reshape(128, k * cols)
        .astype(np_dt, copy=False)
    )


def _pack_inputs(x, gate_w, bias, Wg, Wu, Wd, sWg, sWu, sWd):
    import ml_dtypes

    bf16 = ml_dtypes.bfloat16
    x = np.asarray(x, np.float32)
    gate_w = np.asarray(gate_w, np.float32)
    bias = np.asarray(bias, np.float32)
    Wg, Wu, Wd = (np.asarray(a, np.float32) for a in (Wg, Wu, Wd))
    sWg, sWu, sWd = (np.asarray(a, np.float32) for a in (sWg, sWu, sWd))

    ffn = [(sWg, sWu, sWd)] + [(Wg[e], Wu[e], Wd[e]) for e in HOT]
    wg_pack = np.empty((NCHG, 128, KD * 128), bf16)
    wu_pack = np.empty((NCHG, 128, KD * 128), bf16)
    wd_pack = np.empty((NCHG, 128, D), bf16)
    for f, (wgf, wuf, wdf) in enumerate(ffn):
        wgT = np.ascontiguousarray(wgf.T)  # [D, DFF]
        wuT = np.ascontiguousarray(wuf.T)
        wdT = np.ascontiguousarray(wdf.T)  # [DFF, D]
        for p in range(DFF // 128):
            ch = f * (DFF // 128) + p
            wg_pack[ch] = _pack_sbuf16(wgT[:, 128 * p : 128 * (p + 1)], 128, bf16)
            wu_pack[ch] = _pack_sbuf16(wuT[:, 128 * p : 128 * (p + 1)], 128, bf16)
            wd_pack[ch] = wdT[128 * p : 128 * (p + 1), :].astype(bf16)

    gwt = _pack_sbuf16(np.ascontiguousarray(gate_w.T), E, np.float32)
    biasr = np.broadcast_to(np.tile(bias, NT), (128, NT * E)).astype(np.float32)

    in_maps = []
    for c in range(N_CORES):
        g, r = c // DG, c % DG
        xh = np.ascontiguousarray(x[TT * g : TT * (g + 1)].T)  # [D, TT]
        sel = np.zeros((8, NCH * 128), bf16)
        for j in range(NCH):
            f = (NCH * r + j) // (DFF // 128)
            sel[6 if f == 0 else f - 1, 128 * j : 128 * (j + 1)] = 1.0
        in_maps.append(
            {
                "xtf": _pack_sbuf16(xh, TT, np.float32),
                "xtb": _pack_sbuf16(xh, TT, bf16),
                "gwt": gwt,
                "biasr": biasr,
                "selmat": sel,
                "wg": wg_pack[NCH * r : NCH * (r + 1)],
                "wu": wu_pack[NCH * r : NCH * (r + 1)],
                "wd": wd_pack[NCH * r : NCH * (r + 1)],
            }
        )
    return in_maps


def run(inputs, mode=MODE, trace=False):
    nc = build()
    in_maps = _pack_inputs(**inputs)
    res = run_bass_kernel_spmd(
        nc, in_maps, core_ids=list(range(N_CORES)), trace=trace
    )
    out = np.concatenate(
        [res.results[c]["out"].astype(np.float32) for c in range(N_CORES)], axis=0
    )
    return out, res


def kernel(**inputs):
    out, _ = run(inputs, trace=False)
    return out
